# revision 2
# baseline (speedup 1.0000x reference)
"""Additive (Bahdanau) attention on 8 TRN2 NeuronCores, data-parallel over batch.

Reference computation (per batch b):
    q_proj = query @ W1_w.T + W1_b                      # [dim]
    k_proj = key @ W2_w.T + W2_b                        # [S, dim]
    h      = tanh(k_proj + q_proj)                      # [S, dim]
    score  = h @ V_w[0] + V_b                           # [S]   (V_b cancels in softmax)
    attn   = softmax(score)                             # [S]
    context= attn @ value                               # [dim]

Device mapping (per core, B_LOC=4 batches):
  - keyT [4, D, S] bf16 (host-transposed) so the big matmul contracts d on
    partitions: k_projT tile [128 e, 512 s] = sum_d W2T[d,e] . keyT[d,s].
  - tanh fused on ScalarE with per-partition bias qb[e,b] = q_proj[e,b]+W1_b+W2_b.
  - score via M=1 matmuls (lhsT=V column), softmax rows on partition 0,
    unnormalized exp row -> per-s-tile columns via K=1 broadcast matmuls,
    context accumulated over s-tiles, normalized by 1/sumexp in the epilogue.
"""

import os
import numpy as np
import ml_dtypes
from contextlib import ExitStack

import concourse.bass as bass
import concourse.mybir as mybir
import concourse.tile as tile
from concourse import bacc
from concourse.bass import ts
from concourse.bass_utils import run_bass_kernel_spmd

BF16 = mybir.dt.bfloat16
F32 = mybir.dt.float32
AF = mybir.ActivationFunctionType

N_CORES = 8
B, S, D = 32, 2048, 1024          # full problem
B_LOC = B // N_CORES              # 4 batches per core
P = 128                           # partitions
NCH = D // P                      # 8 chunks of 128 along d and e
SW = 512                          # s-strip width (one PSUM bank at f32)
NSTRIP = S // SW                  # 4 strips per batch
STILE = S // P                    # 16 s-tiles of 128

_CACHE = {}
LAST_EXEC_NS = None


def _build_nc():
    nc = bacc.Bacc("TRN2", target_bir_lowering=False, debug=False,
                   num_devices=N_CORES)
    keyT = nc.declare_dram_parameter("keyT", [B_LOC, D, S], BF16, isOutput=False)
    value = nc.declare_dram_parameter("value", [B_LOC, S, D], BF16, isOutput=False)
    w2t = nc.declare_dram_parameter("w2t", [D, D], BF16, isOutput=False)
    w1t = nc.declare_dram_parameter("w1t", [D, D], BF16, isOutput=False)
    queryT = nc.declare_dram_parameter("queryT", [D, B_LOC], BF16, isOutput=False)
    w1b = nc.declare_dram_parameter("w1b", [D], F32, isOutput=False)
    w2b = nc.declare_dram_parameter("w2b", [D], F32, isOutput=False)
    vw = nc.declare_dram_parameter("vw", [D], BF16, isOutput=False)
    ctx_o = nc.declare_dram_parameter("ctxo", [B_LOC, D], F32, isOutput=True)
    attn_o = nc.declare_dram_parameter("attn", [B_LOC, S], F32, isOutput=True)

    with ExitStack() as ctx:
        tc = ctx.enter_context(tile.TileContext(nc))
        const = ctx.enter_context(tc.tile_pool(name="const", bufs=1))
        kpool = ctx.enter_context(tc.tile_pool(name="kpool", bufs=2))
        hpool = ctx.enter_context(tc.tile_pool(name="hpool", bufs=2))
        vpool = ctx.enter_context(tc.tile_pool(name="vpool", bufs=2))
        rows = ctx.enter_context(tc.tile_pool(name="rows", bufs=2))
        stat = ctx.enter_context(tc.tile_pool(name="stat", bufs=4))
        ecolp = ctx.enter_context(tc.tile_pool(name="ecolp", bufs=4))
        ctxop = ctx.enter_context(tc.tile_pool(name="ctxop", bufs=2))
        psA = ctx.enter_context(tc.tile_pool(name="psA", bufs=2, space="PSUM"))
        psS = ctx.enter_context(tc.tile_pool(name="psS", bufs=2, space="PSUM"))
        psB = ctx.enter_context(tc.tile_pool(name="psB", bufs=2, space="PSUM"))
        psC = ctx.enter_context(tc.tile_pool(name="psC", bufs=1, space="PSUM"))

        # ---- resident weights / constants ----
        w2t_sb = const.tile([P, NCH, D], BF16)
        nc.sync.dma_start(out=w2t_sb, in_=w2t.ap().rearrange("(do di) e -> di do e", di=P))
        w1t_sb = const.tile([P, NCH, D], BF16)
        nc.sync.dma_start(out=w1t_sb, in_=w1t.ap().rearrange("(do di) e -> di do e", di=P))
        q_sb = const.tile([P, NCH, B_LOC], BF16)
        nc.sync.dma_start(out=q_sb, in_=queryT.ap().rearrange("(do di) b -> di do b", di=P))
        w1b_sb = const.tile([P, NCH], F32)
        nc.sync.dma_start(out=w1b_sb, in_=w1b.ap().rearrange("(do di) -> di do", di=P))
        w2b_sb = const.tile([P, NCH], F32)
        nc.sync.dma_start(out=w2b_sb, in_=w2b.ap().rearrange("(do di) -> di do", di=P))
        vw_sb = const.tile([P, NCH], BF16)
        nc.sync.dma_start(out=vw_sb, in_=vw.ap().rearrange("(do di) -> di do", di=P))
        ones_sb = const.tile([1, 1], BF16)
        nc.vector.memset(ones_sb, 1.0)
        biassum = const.tile([P, NCH], F32)
        nc.vector.tensor_add(biassum, w1b_sb, w2b_sb)

        # ---- q_proj + combined tanh bias: qb[e, b] ----
        qb_sb = const.tile([P, NCH, B_LOC], F32)
        for j in range(NCH):
            pq = psB.tile([P, B_LOC], F32, tag="small")
            for i in range(NCH):
                nc.tensor.matmul(pq, lhsT=w1t_sb[:, i, ts(j, P)], rhs=q_sb[:, i, :],
                                 start=(i == 0), stop=(i == NCH - 1))
            nc.vector.tensor_scalar_add(qb_sb[:, j, :], pq, biassum[:, j:j + 1])

        score_sb = [None] * B_LOC

        def emit_A(b):
            score_sb[b] = rows.tile([1, S], F32, tag="score", name=f"score_{b}")
            for t in range(NSTRIP):
                kt = kpool.tile([P, NCH, SW], BF16, tag="kT")
                nc.sync.dma_start(
                    out=kt,
                    in_=keyT.ap()[b].rearrange("(do di) s -> di do s", di=P)[:, :, ts(t, SW)])
                h = hpool.tile([P, NCH, SW], BF16, tag="h")
                for j in range(NCH):
                    pk = psA.tile([P, SW], F32, tag="pk")
                    for i in range(NCH):
                        nc.tensor.matmul(pk, lhsT=w2t_sb[:, i, ts(j, P)], rhs=kt[:, i, :],
                                         start=(i == 0), stop=(i == NCH - 1))
                    nc.scalar.activation(out=h[:, j, :], in_=pk, func=AF.Tanh,
                                         bias=qb_sb[:, j, b:b + 1], scale=1.0)
                ps_s = psS.tile([1, SW], F32, tag="pss")
                for j in range(NCH):
                    nc.tensor.matmul(ps_s, lhsT=vw_sb[:, j:j + 1], rhs=h[:, j, :],
                                     start=(j == 0), stop=(j == NCH - 1))
                nc.vector.tensor_copy(score_sb[b][:, ts(t, SW)], ps_s)

        def emit_BC(b):
            # softmax pieces (unnormalized exp + 1/sum)
            sumexp = stat.tile([1, 1], F32, tag="sum")
            exp_b = rows.tile([1, S], BF16, tag="exp")
            nc.scalar.activation(out=exp_b, in_=score_sb[b], func=AF.Exp,
                                 bias=0.0, scale=1.0, accum_out=sumexp)
            recip = stat.tile([1, 1], F32, tag="recip")
            nc.vector.reciprocal(recip, sumexp)
            attn_sb = rows.tile([1, S], F32, tag="attn")
            nc.vector.tensor_scalar_mul(attn_sb, exp_b, recip)
            nc.sync.dma_start(out=attn_o.ap()[b:b + 1, :], in_=attn_sb)

            # context: accumulate exp-weighted value, scale by recip at the end
            vts = []
            for c in range(NSTRIP):
                vt = vpool.tile([P, NSTRIP, D], BF16, tag="vt")
                nc.sync.dma_start(
                    out=vt,
                    in_=value.ap()[b].rearrange("(ko ki) d -> ki ko d", ki=P)[:, ts(c, NSTRIP), :])
                vts.append(vt)
            pc = psC.tile([1, 2 * SW], F32, tag="pc")
            prev = None
            for k in range(STILE):
                pb = psB.tile([P, 1], F32, tag="small")
                nc.tensor.matmul(pb, lhsT=exp_b[0:1, ts(k, P)], rhs=ones_sb,
                                 start=True, stop=True)
                ec = ecolp.tile([P, 1], BF16, tag="ecol")
                nc.scalar.copy(ec, pb)
                if prev is not None:
                    pk_, ec_ = prev
                    vt = vts[pk_ // NSTRIP]
                    for n in range(2):
                        nc.tensor.matmul(pc[:, ts(n, SW)], lhsT=ec_,
                                         rhs=vt[:, pk_ % NSTRIP, ts(n, SW)],
                                         start=(pk_ == 0), stop=(pk_ == STILE - 1))
                prev = (k, ec)
            pk_, ec_ = prev
            vt = vts[pk_ // NSTRIP]
            for n in range(2):
                nc.tensor.matmul(pc[:, ts(n, SW)], lhsT=ec_,
                                 rhs=vt[:, pk_ % NSTRIP, ts(n, SW)],
                                 start=(pk_ == 0), stop=(pk_ == STILE - 1))
            ctx_sb = ctxop.tile([1, D], F32, tag="ctxo")
            for n in range(2):
                nc.scalar.activation(out=ctx_sb[:, ts(n, SW)], in_=pc[:, ts(n, SW)],
                                     func=AF.Copy, bias=0.0, scale=recip[:, 0:1])
            nc.sync.dma_start(out=ctx_o.ap()[b:b + 1, :], in_=ctx_sb)

        # pipeline: keep PE busy on A(b+1) while B/C(b) waits on softmax
        emit_A(0)
        emit_A(1)
        emit_BC(0)
        emit_A(2)
        emit_BC(1)
        emit_A(3)
        emit_BC(2)
        emit_BC(3)

    nc.compile()
    return nc


def kernel(query, key, value, W1_w, W1_b, W2_w, W2_b, V_w, V_b):
    global LAST_EXEC_NS
    query = np.asarray(query, dtype=np.float32)
    key = np.asarray(key, dtype=np.float32)
    value = np.asarray(value, dtype=np.float32)
    W1_w = np.asarray(W1_w, dtype=np.float32)
    W1_b = np.asarray(W1_b, dtype=np.float32)
    W2_w = np.asarray(W2_w, dtype=np.float32)
    W2_b = np.asarray(W2_b, dtype=np.float32)
    V_w = np.asarray(V_w, dtype=np.float32)

    if "nc" not in _CACHE:
        _CACHE["nc"] = _build_nc()
    nc = _CACHE["nc"]

    bf = ml_dtypes.bfloat16
    w2t = np.ascontiguousarray(W2_w.T).astype(bf)
    w1t = np.ascontiguousarray(W1_w.T).astype(bf)
    vw_ = V_w[0].astype(bf)
    key_bf = key.astype(bf)
    val_bf = value.astype(bf)
    qT = np.ascontiguousarray(query.T).astype(bf)

    in_maps = []
    for c in range(N_CORES):
        sl = slice(B_LOC * c, B_LOC * (c + 1))
        in_maps.append({
            "keyT": np.ascontiguousarray(key_bf[sl].transpose(0, 2, 1)),
            "value": val_bf[sl],
            "w2t": w2t,
            "w1t": w1t,
            "queryT": np.ascontiguousarray(qT[:, sl]),
            "w1b": W1_b,
            "w2b": W2_b,
            "vw": vw_,
        })

    res = run_bass_kernel_spmd(nc, in_maps, core_ids=list(range(N_CORES)))
    LAST_EXEC_NS = res.exec_time_ns

    context = np.concatenate([res.results[c]["ctxo"] for c in range(N_CORES)], axis=0)
    attn = np.concatenate([res.results[c]["attn"] for c in range(N_CORES)], axis=0)
    return (context.astype(np.float32), attn.astype(np.float32))
